# revision 1
# baseline (speedup 1.0000x reference)
"""Distributed TRN2 kernel for nn_CustomFullyConnectedLayerSoftmax.

Math: the reference's scatter-add builds W[r, c] = V_scaled[(r-c) % 2048, c]
(each (r, c) hit exactly once -> pure permutation), then out = x @ W.T.
So out[:, r] needs column r of W.T, i.e. W.T[c, r] = V_scaled[(r-c)%2048, c].

Sharding: output columns r are split across 8 cores (256 each). Core i
receives B_i = W.T[:, 256*i : 256*(i+1)] as a dense [2048, 256] operand,
interleaved with the replicated x.T into a single input tensor laid out in
SBUF geometry: IN[p, k, 0:32] = x.T[k*128+p, :], IN[p, k, 32:288] =
B_i[k*128+p, :]. Each core computes its disjoint out[:, 256*i:256*(i+1)] =
x @ B_i with 16 accumulating matmuls -- no collectives; host concatenates
the 8 slices.

Device traffic per core: its 1/8 share of V plus a replicated x -- the
memory roofline for this op.
"""

import numpy as np

from concourse import bass, bacc, mybir, tile
from concourse import bass_utils

IN_F = 2048
OUT_F = 2048
TOTAL = 2048
BATCH = 32
N_CORES = 8
R_SH = OUT_F // N_CORES          # 256 output columns per core
K_CH = IN_F // 128               # 16 contraction chunks of 128
W_CH = BATCH + R_SH              # 288 = interleaved xT + B row width
K_TOPK = 1844                    # ceil(int(0.9 * 2048 * 2048) / 2048)

# 'f32' or 'bf16' compute/storage dtype for the matmul operands.
DEVICE_DTYPE = "bf16"
# Chunks the load+matmul pipeline is split into (must divide K_CH).
N_SPLITS = 4
# True: raw hand-scheduled bacc kernel; False: Tile-scheduled kernel.
RAW = True
# Keep the end-of-stream wait for the output DMA's completion semaphore.
SAFE_WAIT = True

TRACE = False          # set True (from test.py) to capture neuron-profile
TRACE_KWARGS = {}
LAST_RESULT = None     # BassKernelResults of the most recent run

_graph_cache = {}


def _mybir_dt(key):
    return mybir.dt.float32 if key == "f32" else mybir.dt.bfloat16


def _np_dt(key):
    return mybir.dt.np(_mybir_dt(key))


def _build_graph_tile(dtype_key):
    dt = _mybir_dt(dtype_key)
    nc = bacc.Bacc("TRN2", target_bir_lowering=False, debug=False,
                   enable_asserts=False)

    in_d = nc.dram_tensor("IN", [128, K_CH, W_CH], dt, kind="ExternalInput")
    out_d = nc.dram_tensor("out", [BATCH, R_SH], mybir.dt.float32,
                           kind="ExternalOutput")

    kper = K_CH // N_SPLITS
    dma_engines = [nc.sync, nc.scalar]
    with tile.TileContext(nc) as tc:
        with (
            tc.tile_pool(name="inpool", bufs=N_SPLITS) as inpool,
            tc.tile_pool(name="opool", bufs=1) as opool,
            tc.tile_pool(name="psum", bufs=1, space="PSUM") as pspool,
        ):
            acc = pspool.tile([BATCH, R_SH], mybir.dt.float32)
            tiles = []
            for j in range(N_SPLITS):
                t = inpool.tile([128, kper, W_CH], dt, tag="in")
                dma_engines[j % 2].dma_start(
                    t[:], in_d[:, j * kper:(j + 1) * kper, :])
                tiles.append(t)
            for j in range(N_SPLITS):
                for k in range(kper):
                    kk = j * kper + k
                    nc.tensor.matmul(
                        acc[:],
                        tiles[j][:, k, 0:BATCH],
                        tiles[j][:, k, BATCH:W_CH],
                        start=(kk == 0),
                        stop=(kk == K_CH - 1),
                    )
            ot = opool.tile([BATCH, R_SH], mybir.dt.float32)
            nc.vector.tensor_copy(ot[:], acc[:])
            nc.sync.dma_start(out_d[:], ot[:])

    nc.compile()
    return nc


# k-slice counts per pipelined chunk (must sum to K_CH). Small first chunk
# gets the PE started early; small last chunk minimizes the matmul tail
# exposed after the final DMA-completion semaphore.
CHUNKS = [3, 4, 4, 5]
# How many DMA-issue engines to spread input chunks across (2 or 3).
N_DMA_ENGINES = 2
# Optional explicit per-chunk engine assignment (overrides round-robin).
ENG_PATTERN = None
# Dummy matmuls issued into a scratch PSUM bank while input DMAs stream,
# to lift the PE out of its cold HAM throttle (213ns -> ~107ns per MM)
# before the real matmuls run. 0 disables.
WARMUP_MMS = 16


def _build_graph_raw(dtype_key):
    dt = _mybir_dt(dtype_key)
    nc = bass.Bass("TRN2", target_bir_lowering=False, debug=False,
                   enable_asserts=False)

    in_d = nc.dram_tensor("IN", [128, K_CH, W_CH], dt, kind="ExternalInput")
    out_d = nc.dram_tensor("out", [BATCH, R_SH], mybir.dt.float32,
                           kind="ExternalOutput")

    assert sum(CHUNKS) == K_CH
    bounds = [0]
    for c in CHUNKS:
        bounds.append(bounds[-1] + c)
    # chunk j -> issuing engine index (0=sync HWDGE, 1=scalar HWDGE,
    # 2=gpsimd SWDGE)
    if ENG_PATTERN is not None:
        eng_of = list(ENG_PATTERN)
        assert len(eng_of) == len(CHUNKS)
    else:
        eng_of = [j % N_DMA_ENGINES for j in range(len(CHUNKS))]

    import contextlib
    with contextlib.ExitStack() as stack:
        # One semaphore per DMA: exact completion tracking with no
        # assumption about completion ORDER between DMAs on one ring
        # (observed on cold runs: a small DMA queued after a large one can
        # complete first, breaking cumulative-threshold counting).
        csems = [stack.enter_context(nc.semaphore(f"cs{j}"))
                 for j in range(len(CHUNKS))]
        osem = stack.enter_context(nc.semaphore("osem"))
        msem = stack.enter_context(nc.semaphore("msem"))
        psem = stack.enter_context(nc.semaphore("psem"))
        inb = stack.enter_context(
            nc.sbuf_tensor("inb", [128, K_CH, W_CH], dt))
        acc = stack.enter_context(
            nc.psum_tensor("acc", [BATCH, R_SH], mybir.dt.float32))
        if WARMUP_MMS:
            warm = stack.enter_context(
                nc.psum_tensor("warm", [BATCH, R_SH], mybir.dt.float32))
        ot = stack.enter_context(
            nc.sbuf_tensor("ot", [BATCH, R_SH], mybir.dt.float32))
        block = stack.enter_context(nc.Block())

        # Even chunks stream through sync's HWDGE ring, odd through scalar's.
        @block.sync
        def _(sync):
            for j in range(len(CHUNKS)):
                if eng_of[j] == 0:
                    sync.dma_start(
                        inb[:, bounds[j]:bounds[j + 1], :],
                        in_d[:, bounds[j]:bounds[j + 1], :],
                    ).then_inc(csems[j], 16)
            sync.wait_ge(psem, 1)
            sync.dma_start(out_d[:, :], ot[:, :]).then_inc(osem, 16)
            # The host reads `out` right after NEFF completion; the output
            # DMA must be complete before this engine stream ends.
            if SAFE_WAIT:
                sync.wait_ge(osem, 16)

        @block.scalar
        def _(scalar):
            for j in range(len(CHUNKS)):
                if eng_of[j] == 1:
                    scalar.dma_start(
                        inb[:, bounds[j]:bounds[j + 1], :],
                        in_d[:, bounds[j]:bounds[j + 1], :],
                    ).then_inc(csems[j], 16)

        if any(e == 2 for e in eng_of):
            @block.gpsimd
            def _(gpsimd):
                for j in range(len(CHUNKS)):
                    if eng_of[j] == 2:
                        gpsimd.dma_start(
                            inb[:, bounds[j]:bounds[j + 1], :],
                            in_d[:, bounds[j]:bounds[j + 1], :],
                        ).then_inc(csems[j], 16)

        @block.tensor
        def _(tensor):
            # Warm-up: PE churns on whatever is in SBUF (result discarded)
            # so the HAM throttle lifts before the real matmuls arrive.
            for _ in range(WARMUP_MMS):
                tensor.matmul(
                    warm[:, :],
                    inb[:, 0, 0:BATCH],
                    inb[:, 0, BATCH:W_CH],
                    start=True,
                    stop=True,
                )
            for j in range(len(CHUNKS)):
                tensor.wait_ge(csems[j], 16)
                for kk in range(bounds[j], bounds[j + 1]):
                    mm = tensor.matmul(
                        acc[:, :],
                        inb[:, kk, 0:BATCH],
                        inb[:, kk, BATCH:W_CH],
                        start=(kk == 0),
                        stop=(kk == K_CH - 1),
                    )
            mm.then_inc(msem, 1)

        @block.vector
        def _(vector):
            vector.wait_ge(msem, 1)
            vector.tensor_copy(ot[:, :], acc[:, :]).then_inc(psem, 1)

    return nc


def _get_graph(dtype_key):
    key = (dtype_key, RAW, tuple(CHUNKS), SAFE_WAIT, N_DMA_ENGINES,
           tuple(ENG_PATTERN) if ENG_PATTERN else None, WARMUP_MMS)
    if key not in _graph_cache:
        build = _build_graph_raw if RAW else _build_graph_tile
        _graph_cache[key] = build(dtype_key)
    return _graph_cache[key]


def _host_shards(x, V, alpha, dtype_key):
    np_dt = _np_dt(dtype_key)

    a = alpha.astype(np.float64)
    e = np.exp(a - a.max())
    scale = np.clip(K_TOPK * (e / e.sum()), 0.0, 1.0).astype(np.float32)
    Vs = V * scale[:, None]                        # [2048, 2048] f32

    # W.T[c, r] = Vs[(r - c) % 2048, c]; with Vt = Vs.T duplicated along
    # columns, row c of W.T is the window Vt2[c, 2048-c : 4096-c] -> a
    # shear expressible as a strided view of the flat buffer.
    Vt2 = np.concatenate([Vs.T, Vs.T], axis=1)     # [2048, 4096]
    flat = np.ascontiguousarray(Vt2).reshape(-1)
    WT = np.lib.stride_tricks.as_strided(
        flat[TOTAL:], shape=(IN_F, OUT_F),
        strides=((2 * TOTAL - 1) * 4, 4))

    xT = np.ascontiguousarray(x.T)                 # [2048, 32]
    # [128, K_CH, BATCH]
    xT_dev = xT.reshape(K_CH, 128, BATCH).transpose(1, 0, 2)

    in_maps = []
    for i in range(N_CORES):
        Bi = np.asarray(WT[:, i * R_SH:(i + 1) * R_SH])   # [2048, 256]
        Bi_dev = Bi.reshape(K_CH, 128, R_SH).transpose(1, 0, 2)
        merged = np.empty((128, K_CH, W_CH), dtype=np_dt)
        merged[:, :, :BATCH] = xT_dev
        merged[:, :, BATCH:] = Bi_dev
        in_maps.append({"IN": merged})
    return in_maps


def kernel(x, V, alpha):
    global LAST_RESULT
    x = np.asarray(x, dtype=np.float32)
    V = np.asarray(V, dtype=np.float32)
    alpha = np.asarray(alpha, dtype=np.float32)

    in_maps = _host_shards(x, V, alpha, DEVICE_DTYPE)
    nc = _get_graph(DEVICE_DTYPE)
    res = bass_utils.run_bass_kernel_spmd(
        nc, in_maps, core_ids=list(range(N_CORES)),
        trace=TRACE, trace_kwargs=TRACE_KWARGS)
    LAST_RESULT = res
    out = np.concatenate([np.asarray(r["out"]) for r in res.results], axis=1)
    return np.ascontiguousarray(out, dtype=np.float32)

